# revision 60
# baseline (speedup 1.0000x reference)
"""Trainium2 Bass kernel for the NFRNN z0-encoder.

Strategy: data-parallel over batch (4096 -> 8 cores x 512). Per core the
backward-time recurrence runs feature-major ([feat, batch]) with two
independent 256-wide batch chunks so PE/ACT/DVE/GPSIMD pipeline across the
sequential 64 steps. The whole pipeline is bf16 (weights, data slab, hidden
state, elementwise intermediates; PSUM accumulation stays fp32): matmuls run
at full PE rate and the bf16 SBUF-only DVE TensorTensor ops hit the 2x_1p
fast mode. Sigmoids are rewritten as tanh (keeps the loop on one ACT table
set with exp); the z-gate weights are host-negated so the gate tanh yields
tq = -tanh(gz/2) directly, giving the blend p = 0.5*m*(1+tq) and
qneg = (p-1)*x in one scalar_tensor_tensor each, and x <- p*n - qneg.
The n-path is tanh(0.5*(tr*hn + v)) where the PSUM region v = 2*inn + hn is
produced by running the whh_n matmul into two regions (wih_n host-prescaled
by 2), removing a DVE op from the chain. The gate tanh is split r/z so the
r half (on the critical path) retires one ACT slot earlier. Per-step dt
biases of the first flow matmul are folded in as K=1 rank-1 PE
accumulations (PSUM accumulation groups are kept strictly sequential per
bank - interleaving start/stop groups within a bank aborts at runtime).
Softplus of the std head and the final layout transpose happen on the host.
"""
import numpy as np
import ml_dtypes

import concourse.bass as bass
import concourse.mybir as mybir
import concourse.tile as tile
from concourse import bacc
from concourse.bass_utils import run_bass_kernel_spmd

B, T, IN_DIM = 4096, 64, 32
D = 2 * IN_DIM          # 64 data features
REC, HID, LAT = 128, 256, 64
NCORES = 8
BS = B // NCORES        # 512 batch per core
# three pipeline chunks per core, staggered a third of a step apart
CHS = (176, 176, 160)
OFFS = (0, 176, 352)
NCH = 3
HT = T // 2             # data slab packs steps 0:32 on partitions 0:64, 32:64 on 64:128

F32 = mybir.dt.float32
F32R = mybir.dt.float32r
BF16 = mybir.dt.bfloat16
U8 = mybir.dt.uint8
AF = mybir.ActivationFunctionType
ALU = mybir.AluOpType

_CACHE = {}

# filler matmuls emitted at each PE dependency-stall boundary (see
# emit_filler): (count, cols) tuned against the NTFF trace
FILL_WARM = 40
# mixed sizes per stall: wide fillers carry the bulk of the coverage, a
# trailing narrow one keeps the head-of-line cost at the stall's end small
FILL_T1 = [(1, 128)]
FILL_T2 = [(1, 128)]
FILL_T3 = [(1, 512), (1, 128)]
FILL_GRU = [(3, 512), (1, 128)]


def _bcast_ap(row_ap, parts):
    return bass.AP(tensor=row_ap.tensor, offset=row_ap.offset,
                   ap=[[0, parts]] + list(row_ap.ap)[1:])


def _build(flags, reps=1):
    (zb1_0, zb1_1, zb2_0, zb2_1, z_brz, z_bhhn) = flags
    nc = bacc.Bacc(enable_partition_id=False)

    dm = {}
    def din(name, shape, dt):
        dm[name] = nc.dram_tensor(name, shape, dt, kind="ExternalInput")
        return dm[name]

    dat_dm = din("dat", [128, HT * BS], BF16)
    msk_dm = din("msk", [T, BS], BF16)
    for l in range(2):
        din(f"w0aT{l}", [128, 256], BF16)
        din(f"w1T{l}", [128, 512], BF16)
        din(f"w2sT{l}", [128, 256], BF16)
        din(f"d0a{l}", [128, T], F32)
        din(f"d0b{l}", [128, T], F32)
        din(f"tsc{l}", [128, T], F32)
        din(f"tsh{l}", [128, T], F32)
        din(f"eb{l}", [128, T], F32)
        din(f"b1t{l}", [128, 2], F32)
        din(f"sb2t{l}", [128, T], F32)
    din("wihT", [128, 384], BF16)
    din("whhT", [128, 384], BF16)
    din("brzt", [128, 2], F32)
    din("biasn", [128, 1], F32)
    din("bhhn", [1, 128], BF16)
    din("ones", [128, 512], BF16)
    din("z0w0T", [128, 100], BF16)
    din("z0b0", [100, 1], F32)
    din("z0w1T", [100, 128], BF16)
    din("z0b1", [128, 1], F32)
    out_dm = nc.dram_tensor("out", [128, BS], F32, kind="ExternalOutput")

    with tile.TileContext(nc) as tc:
        with tc.tile_pool(name="const", bufs=1) as cp, \
             tc.tile_pool(name="shared", bufs=3) as shp, \
             tc.tile_pool(name="sb0", bufs=2) as sb0, \
             tc.tile_pool(name="sb1", bufs=2) as sb1, \
             tc.tile_pool(name="sb2", bufs=2) as sb2, \
             tc.tile_pool(name="ps0", bufs=1, space="PSUM") as ps0, \
             tc.tile_pool(name="ps1", bufs=1, space="PSUM") as ps1, \
             tc.tile_pool(name="ps2", bufs=1, space="PSUM") as ps2, \
             tc.tile_pool(name="ps3", bufs=1, space="PSUM") as ps3, \
             tc.tile_pool(name="psf", bufs=1, space="PSUM") as psf:

            # ---- preload constants ----
            def load(name, shape, dt):
                t = cp.tile(shape, dt, tag=name)
                nc.sync.dma_start(out=t, in_=dm[name][tuple(slice(0, s) for s in shape)])
                return t

            # ones loads first: the HAM-warming filler matmuls read it, and
            # they start as soon as this DMA lands
            ones = load("ones", [128, 512], BF16)
            Tf = psf.tile([128, 512], F32, tag="Tf")

            def emit_filler(spec):
                # dependency-free matmuls into a scratch PSUM bank: keeps the
                # PE array busy through ACT/DVE-bound stretches so the HAM
                # clock gate holds K=8/8 (2.4 GHz) instead of dropping to 1.2
                if isinstance(spec, int):
                    spec = [(spec, 512)]
                elif isinstance(spec, tuple):
                    spec = [spec]
                for n, cols in spec:
                    for _ in range(n):
                        nc.tensor.matmul(Tf[:, 0:cols], ones[:, 0:128],
                                         ones[:, 0:cols], start=True, stop=True)

            dat = cp.tile([128, HT * BS], BF16, tag="dat")
            HB = HT * BS // 2
            # steps 0-3 of the data slab load before the weights so step 0
            # can start as soon as the (smaller) weight DMAs land; the
            # remaining ~4MB queues after everything step-0 needs
            PRE = 4 * BS
            nc.sync.dma_start(out=dat[0:64, 0:PRE], in_=dm["dat"][0:64, 0:PRE])
            lay = []
            for l in range(2):
                lay.append(dict(
                    w0aT=load(f"w0aT{l}", [128, 256], BF16),
                    w1T=load(f"w1T{l}", [128, 512], BF16),
                    w2sT=load(f"w2sT{l}", [128, 256], BF16),
                    d0a=load(f"d0a{l}", [128, T], F32),
                    d0b=load(f"d0b{l}", [128, T], F32),
                    tsc=load(f"tsc{l}", [128, T], F32),
                    tsh=load(f"tsh{l}", [128, T], F32),
                    eb=load(f"eb{l}", [128, T], F32),
                    b1t=load(f"b1t{l}", [128, 2], F32),
                    sb2t=load(f"sb2t{l}", [128, T], F32),
                ))
            wihT = load("wihT", [128, 384], BF16)
            whhT = load("whhT", [128, 384], BF16)
            brzt = load("brzt", [128, 2], F32)
            biasn = load("biasn", [128, 1], F32)
            bhhn = load("bhhn", [1, 128], BF16)
            z0w0T = load("z0w0T", [128, 100], BF16)
            z0b0 = load("z0b0", [100, 1], F32)
            z0w1T = load("z0w1T", [100, 128], BF16)
            z0b1 = load("z0b1", [128, 1], F32)
            mreps = {}

            def get_mrep(s):
                if s not in mreps:
                    mrep = shp.tile([128, BS], BF16, tag="mrep")
                    nc.sync.dma_start(out=mrep[:, :],
                                      in_=_bcast_ap(msk_dm[s % T:s % T + 1, :], 128))
                    mreps[s] = mrep
                    if s - 2 in mreps:
                        del mreps[s - 2]
                return mreps[s]

            # masks for the first steps must beat the bulk data DMAs into the
            # sync queue, or step 0's gru waits ~30us behind them
            get_mrep(0)
            get_mrep(1)
            nc.sync.dma_start(out=dat[0:64, PRE:HB], in_=dm["dat"][0:64, PRE:HB])
            nc.sync.dma_start(out=dat[0:64, HB:2 * HB], in_=dm["dat"][0:64, HB:2 * HB])
            nc.sync.dma_start(out=dat[64:128, 0:HB], in_=dm["dat"][64:128, 0:HB])
            nc.sync.dma_start(out=dat[64:128, HB:2 * HB],
                              in_=dm["dat"][64:128, HB:2 * HB])
            out_sb = cp.tile([128, BS], F32, tag="out_sb")

            # warm the HAM clock gate and keep PE busy through the preload
            # DMAs (the filler stream starts as soon as `ones` lands)
            emit_filler(FILL_WARM)

            # persistent hidden state per chunk
            xs = []
            for c in range(NCH):
                x = cp.tile([128, CHS[c]], BF16, tag=f"x{c}")
                nc.vector.memzero(x[:, :])
                xs.append(x)

            zb1 = (zb1_0, zb1_1)
            zb2 = (zb2_0, zb2_1)
            sbp_ = (sb0, sb1, sb2)[:NCH]
            psp_ = (ps0, ps1, ps2)[:NCH]

            psum_t = {}

            def emit_flow(c, s_rep):
                    s = s_rep % T
                    sbp, psp, x = sbp_[c], psp_[c], xs[c]
                    CW, OFF = CHS[c], OFFS[c]
                    part0 = (s // HT) * 64
                    col0 = (s % HT) * BS + OFF
                    xt = dat[part0:part0 + 64, col0:col0 + CW]

                    T1 = psp.tile([128, 2 * CW], F32, tag="T1")
                    T2 = psp.tile([128, 2 * CW], F32, tag="T2", bufs=(2 if NCH == 2 else 1))
                    # all chunks' T3 share one PSUM bank (column slices)
                    T3all = ps3.tile([128, BS], F32, tag="T3all")
                    T3 = T3all[:, OFF:OFF + CW]

                    for l in range(2):
                        L = lay[l]
                        a0 = 0 if l == 0 else 64    # active (masked-in) dims
                        u0 = 64 - a0                # updated dims
                        rhs1 = x[a0:a0 + 64, :]
                        nc.tensor.matmul(T1[:, 0:CW], L["w0aT"][a0:a0 + 64, 0:128],
                                         rhs1, start=True, stop=True)
                        nc.tensor.matmul(T1[:, CW:2 * CW], L["w0aT"][a0:a0 + 64, 128:256],
                                         rhs1, start=True, stop=True)
                        emit_filler(FILL_T1)
                        # per-step dt bias rides the tanh's per-partition bias
                        # operand (two halves = two bias columns)
                        h1 = sbp.tile([128, 2 * CW], BF16, tag="h1")
                        nc.scalar.activation(h1[:, 0:CW], T1[:, 0:CW], AF.Tanh,
                                             bias=L["d0a"][:, s:s + 1])
                        nc.scalar.activation(h1[:, CW:2 * CW], T1[:, CW:2 * CW], AF.Tanh,
                                             bias=L["d0b"][:, s:s + 1])
                        for mj in range(2):
                            for ki in range(2):
                                nc.tensor.matmul(
                                    T2[:, mj * CW:(mj + 1) * CW],
                                    L["w1T"][:, (2 * ki + mj) * 128:(2 * ki + mj) * 128 + 128],
                                    h1[:, ki * CW:(ki + 1) * CW],
                                    start=(ki == 0), stop=(ki == 1))
                        emit_filler(FILL_T2)
                        h2 = sbp.tile([128, 2 * CW], BF16, tag="h2")
                        if zb1[l]:
                            nc.scalar.activation(h2[:, :], T2[:, :], AF.Tanh)
                        else:
                            nc.scalar.activation(h2[:, 0:CW], T2[:, 0:CW], AF.Tanh,
                                                 bias=L["b1t"][:, 0:1])
                            nc.scalar.activation(h2[:, CW:2 * CW], T2[:, CW:2 * CW], AF.Tanh,
                                                 bias=L["b1t"][:, 1:2])
                        for ki in range(2):
                            nc.tensor.matmul(T3[:, :], L["w2sT"][:, ki * 128:(ki + 1) * 128],
                                             h2[:, ki * CW:(ki + 1) * CW],
                                             start=(ki == 0), stop=(ki == 1))
                        emit_filler(FILL_T3)
                        e = sbp.tile([128, CW], BF16, tag="e")
                        nc.scalar.activation(e[u0:u0 + 64, :], T3[0:64, :], AF.Exp,
                                             bias=L["eb"][u0:u0 + 64, s:s + 1],
                                             scale=L["tsc"][u0:u0 + 64, s:s + 1])
                        # sh = shift*t_shift runs on DVE in parallel with the
                        # exp on ACT; the post-e chain is then two fast
                        # SBUF-only bf16 TensorTensor ops instead of a
                        # PSUM-read scalar_tensor_tensor
                        sh = sbp.tile([128, CW], BF16, tag="sh")
                        nc.vector.tensor_scalar_mul(sh[u0:u0 + 64, :], T3[64:128, :],
                                                    L["tsh"][u0:u0 + 64, s:s + 1])
                        xe = sbp.tile([128, CW], BF16, tag="xe")
                        nc.vector.tensor_mul(xe[u0:u0 + 64, :], x[u0:u0 + 64, :],
                                             e[u0:u0 + 64, :])
                        nc.vector.tensor_add(x[u0:u0 + 64, :], xe[u0:u0 + 64, :],
                                             sh[u0:u0 + 64, :])
                        if not zb2[l]:
                            nc.vector.tensor_scalar_add(x[u0:u0 + 64, :], x[u0:u0 + 64, :],
                                                        L["sb2t"][u0:u0 + 64, s:s + 1])

                    psum_t[c] = (T1, T2, T3, part0, col0)

            def emit_gru(c, s_rep):
                    s = s_rep % T
                    mrep = get_mrep(s_rep)
                    sbp, psp, x = sbp_[c], psp_[c], xs[c]
                    CW, OFF = CHS[c], OFFS[c]
                    T1, T2, T3, part0, col0 = psum_t[c]
                    xt = dat[part0:part0 + 64, col0:col0 + CW]
                    # ---- GRU cell ----
                    wr = slice(part0, part0 + 64)
                    nc.tensor.matmul(T2[:, 0:CW], wihT[wr, 0:128], xt, start=True, stop=False)
                    nc.tensor.matmul(T2[:, 0:CW], whhT[:, 0:128], x[:, :], start=False, stop=True)
                    nc.tensor.matmul(T2[:, CW:2 * CW], wihT[wr, 128:256], xt, start=True, stop=False)
                    nc.tensor.matmul(T2[:, CW:2 * CW], whhT[:, 128:256], x[:, :], start=False, stop=True)
                    if z_bhhn:
                        nc.tensor.matmul(T1[:, CW:2 * CW], whhT[:, 256:384], x[:, :],
                                         start=True, stop=True)
                    else:
                        nc.tensor.matmul(T1[:, CW:2 * CW], whhT[:, 256:384], x[:, :],
                                         start=True, stop=False)
                        nc.tensor.matmul(T1[:, CW:2 * CW], bhhn[0:1, :], ones[0:1, 0:CW],
                                         start=False, stop=True)
                    nc.tensor.matmul(T1[:, 0:CW], wihT[wr, 256:384], xt, start=True, stop=False)
                    if not z_bhhn:
                        nc.tensor.matmul(T1[:, 0:CW], bhhn[0:1, :], ones[0:1, 0:CW],
                                         start=False, stop=False)
                    nc.tensor.matmul(T1[:, 0:CW], whhT[:, 256:384], x[:, :],
                                     start=False, stop=True)
                    emit_filler(FILL_GRU)

                    trz = sbp.tile([128, 2 * CW], BF16, tag="trz")
                    if z_brz:
                        nc.scalar.activation(trz[:, 0:CW], T2[:, 0:CW], AF.Tanh, scale=0.5)
                        nc.scalar.activation(trz[:, CW:2 * CW], T2[:, CW:2 * CW],
                                             AF.Tanh, scale=0.5)
                    else:
                        nc.scalar.activation(trz[:, 0:CW], T2[:, 0:CW], AF.Tanh,
                                             scale=0.5, bias=brzt[:, 0:1])
                        nc.scalar.activation(trz[:, CW:2 * CW], T2[:, CW:2 * CW], AF.Tanh,
                                             scale=0.5, bias=brzt[:, 1:2])
                    tz = trz[:, CW:2 * CW]
                    wge = sbp.tile([128, CW], BF16, tag="wge")
                    nc.vector.tensor_mul(wge[:, :], trz[:, 0:CW], T1[:, CW:2 * CW])
                    npre = sbp.tile([128, CW], BF16, tag="npre")
                    nc.vector.tensor_add(npre[:, :], wge[:, :], T1[:, 0:CW])
                    ng = sbp.tile([128, CW], BF16, tag="ng")
                    nc.scalar.activation(ng[:, :], npre[:, :], AF.Tanh, scale=0.5,
                                         bias=biasn[:, 0:1])
                    # z weights negated on host: tz slot holds tq = tanh(-gz/2),
                    # p = 0.5*m*(1+tq), qneg = (p-1)*x, x_new = p*ng - qneg
                    p = sbp.tile([128, CW], BF16, tag="p")
                    nc.vector.scalar_tensor_tensor(p[:, :], tz, 1.0,
                                                   mrep[:, OFF:OFF + CW],
                                                   ALU.add, ALU.mult)
                    qneg = sbp.tile([128, CW], BF16, tag="qneg")
                    nc.vector.scalar_tensor_tensor(qneg[:, :], p[:, :], 1.0,
                                                   x[:, :], ALU.subtract, ALU.mult)
                    u1 = sbp.tile([128, CW], BF16, tag="u1")
                    nc.vector.tensor_mul(u1[:, :], p[:, :], ng[:, :])
                    nc.vector.tensor_tensor(x[:, :], u1[:, :], qneg[:, :],
                                            ALU.subtract)

            def emit_head(c):
                sbp, psp, x = sbp_[c], psp_[c], xs[c]
                CW, OFF = CHS[c], OFFS[c]
                T3all = ps3.tile([128, BS], F32, tag="T3all")
                hd1 = T3all[0:100, OFF:OFF + CW]
                nc.tensor.matmul(hd1[:, :], z0w0T[:, :], x[:, :], start=True, stop=True)
                hid = sbp.tile([100, CW], BF16, tag="h1")
                nc.scalar.activation(hid[:, :], hd1[:, :], AF.Tanh, bias=z0b0[:, 0:1])
                hd2 = psp.tile([128, CW], F32, tag="T1")
                nc.tensor.matmul(hd2[:, :], z0w1T[:, :], hid[:, :], start=True, stop=True)
                nc.scalar.activation(out_sb[:, OFF:OFF + CW], hd2[:, :],
                                     AF.Identity, bias=z0b1[:, 0:1])

            N = reps * T
            if NCH == 2:
                for s_rep in range(N + 1):
                    if s_rep < N:
                        emit_flow(0, s_rep)
                    if s_rep >= 1:
                        if s_rep == N:
                            emit_head(0)  # overlaps chunk 1's final gru chain
                        emit_gru(1, s_rep - 1)
                    if s_rep < N:
                        emit_gru(0, s_rep)
                        emit_flow(1, s_rep)
                emit_head(1)
            else:
                # three-phase rotation: every consumer gets two emission slots
                # of other-chunk work between it and its producer
                for s_rep in range(N):
                    emit_flow(0, s_rep)
                    if s_rep >= 1:
                        emit_gru(2, s_rep - 1)
                    emit_flow(1, s_rep)
                    emit_gru(0, s_rep)
                    emit_flow(2, s_rep)
                    emit_gru(1, s_rep)
                emit_head(0)
                emit_gru(2, N - 1)
                emit_head(1)
                emit_head(2)
            nc.sync.dma_start(out=out_dm[:, :], in_=out_sb[:, :])

    nc.compile()
    return nc


def _prep(inputs):
    f = {k: np.asarray(v, dtype=np.float32) for k, v in inputs.items()}
    data, ts = f["data"], f["time_steps"]
    dts = np.concatenate([np.float32([-0.01]), (ts[:-1] - ts[1:])[::-1]]).astype(np.float32)

    shared = {}
    flags = []
    for l in range(2):
        w0, b0 = f[f"f{l}_w0"], f[f"f{l}_b0"]
        w1, b1 = f[f"f{l}_w1"], f[f"f{l}_b1"]
        w2, b2 = f[f"f{l}_w2"], f[f"f{l}_b2"]
        tw = f[f"f{l}_tw"]
        act = slice(0, 64) if l == 0 else slice(64, 128)
        upd = slice(64, 128) if l == 0 else slice(0, 64)

        w0a = w0[:, act]                       # [256, 64]
        w0aT = np.ascontiguousarray(w0a.T)     # [64, 256]
        shared[f"w0aT{l}"] = np.concatenate([w0aT, w0aT], axis=0)  # [128,256] dup
        w1T = np.ascontiguousarray(w1.T)       # [256, 256] (in,out)
        blk = np.empty((128, 512), np.float32)
        for ki in range(2):
            for mj in range(2):
                blk[:, (2 * ki + mj) * 128:(2 * ki + mj) * 128 + 128] = \
                    w1T[ki * 128:(ki + 1) * 128, mj * 128:(mj + 1) * 128]
        shared[f"w1T{l}"] = blk
        ui = np.arange(128)[upd]
        w2s = np.concatenate([w2[ui, :], w2[128 + ui, :]], axis=0)  # [128, 256]
        b2s = np.concatenate([b2[ui], b2[128 + ui]])
        w2sT = np.ascontiguousarray(w2s.T)     # [256, 128]
        shared[f"w2sT{l}"] = np.concatenate([w2sT[0:128, :].reshape(128, 128),
                                             w2sT[128:256, :].reshape(128, 128)], axis=1)
        d0 = np.outer(w0[:, 128], dts) + b0[:, None]       # [256, T]
        shared[f"d0a{l}"] = np.ascontiguousarray(d0[0:128, :])
        shared[f"d0b{l}"] = np.ascontiguousarray(d0[128:256, :])
        tt = np.tanh(np.outer(tw, dts))                    # [256, T]
        tscl = np.ascontiguousarray(tt[0:128][ui, :])    # [64, T]
        tshl = np.ascontiguousarray(tt[128:256][ui, :])  # [64, T]
        ebl = tscl * b2s[0:64, None]
        sb2l = tshl * b2s[64:128, None]
        dup = lambda v: np.ascontiguousarray(np.concatenate([v, v], axis=0))
        shared[f"tsc{l}"] = dup(tscl)
        shared[f"tsh{l}"] = dup(tshl)
        shared[f"eb{l}"] = dup(ebl)
        shared[f"sb2t{l}"] = dup(sb2l)
        b1t = np.stack([b1[0:128], b1[128:256]], axis=1)
        shared[f"b1t{l}"] = np.ascontiguousarray(b1t)
        flags += [bool(np.all(b1 == 0)), bool(np.all(b2 == 0))]

    wih, whh = f["gru_wih"], f["gru_whh"]
    bih, bhh = f["gru_bih"], f["gru_bhh"]
    wihT = np.ascontiguousarray(wih.T)                     # [64, 384]
    shared["wihT"] = np.concatenate([wihT, wihT], axis=0)  # [128, 384]
    shared["whhT"] = np.ascontiguousarray(whh.T)           # [128, 384]
    wihT = shared["wihT"].copy(); whhT = shared["whhT"].copy()
    wihT[:, 128:256] *= -1.0
    whhT[:, 128:256] *= -1.0
    wihT[:, 256:384] *= 2.0
    shared["wihT"] = wihT; shared["whhT"] = whhT
    brz = 0.5 * (bih[0:256] + bhh[0:256])
    brz = np.concatenate([brz[0:128], -brz[128:256]])
    shared["brzt"] = np.ascontiguousarray(np.stack([brz[0:128], brz[128:256]], axis=1))
    shared["biasn"] = np.ascontiguousarray(bih[256:384][:, None])
    shared["bhhn"] = np.ascontiguousarray(bhh[256:384][None, :])
    shared["ones"] = np.ones((128, 512), np.float32)
    shared["z0w0T"] = np.ascontiguousarray(f["z0_w0"].T)   # [128, 100]
    shared["z0b0"] = np.ascontiguousarray(f["z0_b0"][:, None])
    shared["z0w1T"] = np.ascontiguousarray(f["z0_w1"].T)   # [100, 128]
    shared["z0b1"] = np.ascontiguousarray(f["z0_b1"][:, None])
    flags += [bool(np.all(brz == 0)), bool(np.all(bhh[256:384] == 0))]
    # flags order: zb1_0, zb2_0, zb1_1, zb2_1, z_brz, z_bhhn -> reorder
    flags = (flags[0], flags[2], flags[1], flags[3], flags[4], flags[5])

    # data slab [D, T_rev, B] and masks
    arr = np.ascontiguousarray(data.transpose(2, 1, 0)[:, ::-1, :])   # [64, 64, 4096]
    mask = 0.5 * (data[:, ::-1, IN_DIM:].sum(axis=2) > 0).astype(np.float32)  # [B, T] rev
    mask = np.ascontiguousarray(mask.T)                               # [T, B]

    BF = ml_dtypes.bfloat16
    for k in ("wihT", "whhT", "z0w0T", "z0w1T", "bhhn", "ones"):
        shared[k] = shared[k].astype(BF)
    for l in range(2):
        for k in (f"w0aT{l}", f"w1T{l}", f"w2sT{l}"):
            shared[k] = shared[k].astype(BF)

    in_maps = []
    for c in range(NCORES):
        sl = slice(c * BS, (c + 1) * BS)
        ac = arr[:, :, sl]                                            # [64, 64, 512]
        packed = np.empty((128, HT * BS), np.float32)
        packed[0:64, :] = ac[:, 0:HT, :].reshape(64, HT * BS)
        packed[64:128, :] = ac[:, HT:T, :].reshape(64, HT * BS)
        m = {"dat": packed.astype(BF), "msk": np.ascontiguousarray(mask[:, sl]).astype(BF)}
        m.update(shared)
        in_maps.append(m)
    return in_maps, flags


def kernel(**inputs):
    in_maps, flags = _prep(inputs)
    if _CACHE.get("flags") != flags:
        _CACHE["nc"] = _build(flags)
        _CACHE["flags"] = flags
    res = run_bass_kernel_spmd(_CACHE["nc"], in_maps, core_ids=list(range(NCORES)))
    _CACHE["last_res"] = res
    mean = np.empty((B, LAT), np.float32)
    stdp = np.empty((B, LAT), np.float32)
    for c in range(NCORES):
        o = res.results[c]["out"]                 # [128, 512]
        mean[c * BS:(c + 1) * BS] = o[0:LAT, :].T
        stdp[c * BS:(c + 1) * BS] = o[LAT:2 * LAT, :].T
    std = np.logaddexp(0.0, stdp).astype(np.float32)      # softplus
    return mean[None, :, :], std[None, :, :]



# revision 62
# speedup vs baseline: 1.0358x; 1.0358x over previous
"""Trainium2 Bass kernel for the NFRNN z0-encoder.

Strategy: data-parallel over batch (4096 -> 8 cores x 512). Per core the
backward-time recurrence runs feature-major ([feat, batch]) with two
independent 256-wide batch chunks so PE/ACT/DVE pipeline across the
sequential 64 steps. The whole pipeline is bf16 (weights, data slab, hidden
state, elementwise intermediates; PSUM accumulation stays fp32): matmuls run
at full PE rate and the bf16 SBUF-only DVE TensorTensor ops hit the 2x_1p
fast mode. Sigmoids are rewritten as tanh (keeps the loop on one ACT table
set with exp); the z-gate weights are host-negated so the gate tanh yields
tq = -tanh(gz/2) directly, giving the blend p = 0.5*m*(1+tq) and
qneg = (p-1)*x in one scalar_tensor_tensor each, and x <- p*n - qneg.
The n-path is tanh(0.5*(tr*hn + v)) where the PSUM region v = 2*inn + hn is
produced by running the whh_n matmul into two regions (wih_n host-prescaled
by 2), removing a DVE op from the chain. The gate tanh is split r/z so the
r half (on the critical path) retires one ACT slot earlier. Per-step dt
biases of the flow's first matmul ride the h1 tanh's per-partition ACT bias
operand (two bias columns, one per hidden half). The flow epilogue
precomputes sh = shift*t_shift on DVE in parallel with the exp so the
post-exp chain is two fast SBUF-only TensorTensor ops.

The dominant HW effect: the PE HAM clock gate. With the natural instruction
stream the PE idles briefly at every cross-engine dependency, the HAM
re-throttles the array to K=4/8 (1.2 GHz) a few microseconds in, and every
matmul runs at half rate for the rest of the kernel. Dependency-free
"filler" matmuls into a dedicated scratch PSUM bank are emitted at each
known stall boundary (sized per stall, wide fillers for coverage plus a
trailing narrow one to bound head-of-line delay) plus a warmup burst during
the preload DMAs; this holds K=8/8 for ~0.8ms of the ~1.2ms run.
PSUM accumulation groups are kept strictly sequential per bank -
interleaving start/stop groups within a bank aborts at runtime.
Softplus of the std head and the final layout transpose happen on the host.
"""
import numpy as np
import ml_dtypes

import concourse.bass as bass
import concourse.mybir as mybir
import concourse.tile as tile
from concourse import bacc
from concourse.bass_utils import run_bass_kernel_spmd

B, T, IN_DIM = 4096, 64, 32
D = 2 * IN_DIM          # 64 data features
REC, HID, LAT = 128, 256, 64
NCORES = 8
BS = B // NCORES        # 512 batch per core
# two pipeline chunks per core, staggered half a step apart
CHS = (256, 256)
OFFS = (0, 256)
NCH = 2
HT = T // 2             # data slab packs steps 0:32 on partitions 0:64, 32:64 on 64:128

F32 = mybir.dt.float32
F32R = mybir.dt.float32r
BF16 = mybir.dt.bfloat16
U8 = mybir.dt.uint8
AF = mybir.ActivationFunctionType
ALU = mybir.AluOpType

_CACHE = {}

# filler matmuls emitted at each PE dependency-stall boundary (see
# emit_filler): (count, cols) tuned against the NTFF trace
FILL_WARM = 40
# mixed sizes per stall: wide fillers carry the bulk of the coverage, a
# trailing narrow one keeps the head-of-line cost at the stall's end small
FILL_T1 = [(1, 512), (1, 128)]
FILL_T2 = [(1, 512), (1, 128)]
FILL_T3 = [(2, 512), (1, 128)]
FILL_GRU = [(6, 512), (2, 128)]


def _bcast_ap(row_ap, parts):
    return bass.AP(tensor=row_ap.tensor, offset=row_ap.offset,
                   ap=[[0, parts]] + list(row_ap.ap)[1:])


def _build(flags, reps=1):
    (zb1_0, zb1_1, zb2_0, zb2_1, z_brz, z_bhhn) = flags
    nc = bacc.Bacc(enable_partition_id=False)

    dm = {}
    def din(name, shape, dt):
        dm[name] = nc.dram_tensor(name, shape, dt, kind="ExternalInput")
        return dm[name]

    dat_dm = din("dat", [128, HT * BS], BF16)
    msk_dm = din("msk", [T, BS], BF16)
    for l in range(2):
        din(f"w0aT{l}", [128, 256], BF16)
        din(f"w1T{l}", [128, 512], BF16)
        din(f"w2sT{l}", [128, 256], BF16)
        din(f"d0a{l}", [128, T], F32)
        din(f"d0b{l}", [128, T], F32)
        din(f"tsc{l}", [128, T], F32)
        din(f"tsh{l}", [128, T], F32)
        din(f"eb{l}", [128, T], F32)
        din(f"b1t{l}", [128, 2], F32)
        din(f"sb2t{l}", [128, T], F32)
    din("wihT", [128, 384], BF16)
    din("whhT", [128, 384], BF16)
    din("brzt", [128, 2], F32)
    din("biasn", [128, 1], F32)
    din("bhhn", [1, 128], BF16)
    din("ones", [128, 512], BF16)
    din("z0w0T", [128, 100], BF16)
    din("z0b0", [100, 1], F32)
    din("z0w1T", [100, 128], BF16)
    din("z0b1", [128, 1], F32)
    out_dm = nc.dram_tensor("out", [128, BS], F32, kind="ExternalOutput")

    with tile.TileContext(nc) as tc:
        with tc.tile_pool(name="const", bufs=1) as cp, \
             tc.tile_pool(name="shared", bufs=3) as shp, \
             tc.tile_pool(name="sb0", bufs=2) as sb0, \
             tc.tile_pool(name="sb1", bufs=2) as sb1, \
             tc.tile_pool(name="sb2", bufs=2) as sb2, \
             tc.tile_pool(name="ps0", bufs=1, space="PSUM") as ps0, \
             tc.tile_pool(name="ps1", bufs=1, space="PSUM") as ps1, \
             tc.tile_pool(name="ps2", bufs=1, space="PSUM") as ps2, \
             tc.tile_pool(name="ps3", bufs=1, space="PSUM") as ps3, \
             tc.tile_pool(name="psf", bufs=1, space="PSUM") as psf:

            # ---- preload constants ----
            def load(name, shape, dt):
                t = cp.tile(shape, dt, tag=name)
                nc.sync.dma_start(out=t, in_=dm[name][tuple(slice(0, s) for s in shape)])
                return t

            # ones loads first: the HAM-warming filler matmuls read it, and
            # they start as soon as this DMA lands
            ones = load("ones", [128, 512], BF16)
            Tf = psf.tile([128, 512], F32, tag="Tf")

            def emit_filler(spec):
                # dependency-free matmuls into a scratch PSUM bank: keeps the
                # PE array busy through ACT/DVE-bound stretches so the HAM
                # clock gate holds K=8/8 (2.4 GHz) instead of dropping to 1.2
                if isinstance(spec, int):
                    spec = [(spec, 512)]
                elif isinstance(spec, tuple):
                    spec = [spec]
                for n, cols in spec:
                    for _ in range(n):
                        nc.tensor.matmul(Tf[:, 0:cols], ones[:, 0:128],
                                         ones[:, 0:cols], start=True, stop=True)

            dat = cp.tile([128, HT * BS], BF16, tag="dat")
            HB = HT * BS // 2
            # steps 0-3 of the data slab load before the weights so step 0
            # can start as soon as the (smaller) weight DMAs land; the
            # remaining ~4MB queues after everything step-0 needs
            PRE = 4 * BS
            nc.sync.dma_start(out=dat[0:64, 0:PRE], in_=dm["dat"][0:64, 0:PRE])
            lay = []
            for l in range(2):
                lay.append(dict(
                    w0aT=load(f"w0aT{l}", [128, 256], BF16),
                    w1T=load(f"w1T{l}", [128, 512], BF16),
                    w2sT=load(f"w2sT{l}", [128, 256], BF16),
                    d0a=load(f"d0a{l}", [128, T], F32),
                    d0b=load(f"d0b{l}", [128, T], F32),
                    tsc=load(f"tsc{l}", [128, T], F32),
                    tsh=load(f"tsh{l}", [128, T], F32),
                    eb=load(f"eb{l}", [128, T], F32),
                    b1t=load(f"b1t{l}", [128, 2], F32),
                    sb2t=load(f"sb2t{l}", [128, T], F32),
                ))
            wihT = load("wihT", [128, 384], BF16)
            whhT = load("whhT", [128, 384], BF16)
            brzt = load("brzt", [128, 2], F32)
            biasn = load("biasn", [128, 1], F32)
            bhhn = load("bhhn", [1, 128], BF16)
            z0w0T = load("z0w0T", [128, 100], BF16)
            z0b0 = load("z0b0", [100, 1], F32)
            z0w1T = load("z0w1T", [100, 128], BF16)
            z0b1 = load("z0b1", [128, 1], F32)
            mreps = {}

            def get_mrep(s):
                if s not in mreps:
                    mrep = shp.tile([128, BS], BF16, tag="mrep")
                    nc.sync.dma_start(out=mrep[:, :],
                                      in_=_bcast_ap(msk_dm[s % T:s % T + 1, :], 128))
                    mreps[s] = mrep
                    if s - 2 in mreps:
                        del mreps[s - 2]
                return mreps[s]

            # masks for the first steps must beat the bulk data DMAs into the
            # sync queue, or step 0's gru waits ~30us behind them
            get_mrep(0)
            get_mrep(1)
            nc.sync.dma_start(out=dat[0:64, PRE:HB], in_=dm["dat"][0:64, PRE:HB])
            nc.sync.dma_start(out=dat[0:64, HB:2 * HB], in_=dm["dat"][0:64, HB:2 * HB])
            nc.sync.dma_start(out=dat[64:128, 0:HB], in_=dm["dat"][64:128, 0:HB])
            nc.sync.dma_start(out=dat[64:128, HB:2 * HB],
                              in_=dm["dat"][64:128, HB:2 * HB])
            out_sb = cp.tile([128, BS], F32, tag="out_sb")

            # warm the HAM clock gate and keep PE busy through the preload
            # DMAs (the filler stream starts as soon as `ones` lands)
            emit_filler(FILL_WARM)

            # persistent hidden state per chunk
            xs = []
            for c in range(NCH):
                x = cp.tile([128, CHS[c]], BF16, tag=f"x{c}")
                nc.vector.memzero(x[:, :])
                xs.append(x)

            zb1 = (zb1_0, zb1_1)
            zb2 = (zb2_0, zb2_1)
            sbp_ = (sb0, sb1, sb2)[:NCH]
            psp_ = (ps0, ps1, ps2)[:NCH]

            psum_t = {}

            def emit_flow(c, s_rep):
                    s = s_rep % T
                    sbp, psp, x = sbp_[c], psp_[c], xs[c]
                    CW, OFF = CHS[c], OFFS[c]
                    part0 = (s // HT) * 64
                    col0 = (s % HT) * BS + OFF
                    xt = dat[part0:part0 + 64, col0:col0 + CW]

                    T1 = psp.tile([128, 2 * CW], F32, tag="T1")
                    T2 = psp.tile([128, 2 * CW], F32, tag="T2", bufs=2)
                    # all chunks' T3 share one PSUM bank (column slices)
                    T3all = ps3.tile([128, BS], F32, tag="T3all")
                    T3 = T3all[:, OFF:OFF + CW]

                    for l in range(2):
                        L = lay[l]
                        a0 = 0 if l == 0 else 64    # active (masked-in) dims
                        u0 = 64 - a0                # updated dims
                        rhs1 = x[a0:a0 + 64, :]
                        nc.tensor.matmul(T1[:, 0:CW], L["w0aT"][a0:a0 + 64, 0:128],
                                         rhs1, start=True, stop=True)
                        nc.tensor.matmul(T1[:, CW:2 * CW], L["w0aT"][a0:a0 + 64, 128:256],
                                         rhs1, start=True, stop=True)
                        emit_filler(FILL_T1)
                        # per-step dt bias rides the tanh's per-partition bias
                        # operand (two halves = two bias columns)
                        h1 = sbp.tile([128, 2 * CW], BF16, tag="h1")
                        nc.scalar.activation(h1[:, 0:CW], T1[:, 0:CW], AF.Tanh,
                                             bias=L["d0a"][:, s:s + 1])
                        nc.scalar.activation(h1[:, CW:2 * CW], T1[:, CW:2 * CW], AF.Tanh,
                                             bias=L["d0b"][:, s:s + 1])
                        for mj in range(2):
                            for ki in range(2):
                                nc.tensor.matmul(
                                    T2[:, mj * CW:(mj + 1) * CW],
                                    L["w1T"][:, (2 * ki + mj) * 128:(2 * ki + mj) * 128 + 128],
                                    h1[:, ki * CW:(ki + 1) * CW],
                                    start=(ki == 0), stop=(ki == 1))
                        emit_filler(FILL_T2)
                        h2 = sbp.tile([128, 2 * CW], BF16, tag="h2")
                        if zb1[l]:
                            nc.scalar.activation(h2[:, :], T2[:, :], AF.Tanh)
                        else:
                            nc.scalar.activation(h2[:, 0:CW], T2[:, 0:CW], AF.Tanh,
                                                 bias=L["b1t"][:, 0:1])
                            nc.scalar.activation(h2[:, CW:2 * CW], T2[:, CW:2 * CW], AF.Tanh,
                                                 bias=L["b1t"][:, 1:2])
                        for ki in range(2):
                            nc.tensor.matmul(T3[:, :], L["w2sT"][:, ki * 128:(ki + 1) * 128],
                                             h2[:, ki * CW:(ki + 1) * CW],
                                             start=(ki == 0), stop=(ki == 1))
                        emit_filler(FILL_T3)
                        e = sbp.tile([128, CW], BF16, tag="e")
                        nc.scalar.activation(e[u0:u0 + 64, :], T3[0:64, :], AF.Exp,
                                             bias=L["eb"][u0:u0 + 64, s:s + 1],
                                             scale=L["tsc"][u0:u0 + 64, s:s + 1])
                        # sh = shift*t_shift runs on DVE in parallel with the
                        # exp on ACT; the post-e chain is then two fast
                        # SBUF-only bf16 TensorTensor ops instead of a
                        # PSUM-read scalar_tensor_tensor
                        sh = sbp.tile([128, CW], BF16, tag="sh")
                        nc.vector.tensor_scalar_mul(sh[u0:u0 + 64, :], T3[64:128, :],
                                                    L["tsh"][u0:u0 + 64, s:s + 1])
                        xe = sbp.tile([128, CW], BF16, tag="xe")
                        nc.vector.tensor_mul(xe[u0:u0 + 64, :], x[u0:u0 + 64, :],
                                             e[u0:u0 + 64, :])
                        nc.vector.tensor_add(x[u0:u0 + 64, :], xe[u0:u0 + 64, :],
                                             sh[u0:u0 + 64, :])
                        if not zb2[l]:
                            nc.vector.tensor_scalar_add(x[u0:u0 + 64, :], x[u0:u0 + 64, :],
                                                        L["sb2t"][u0:u0 + 64, s:s + 1])

                    psum_t[c] = (T1, T2, T3, part0, col0)

            def emit_gru(c, s_rep):
                    s = s_rep % T
                    mrep = get_mrep(s_rep)
                    sbp, psp, x = sbp_[c], psp_[c], xs[c]
                    CW, OFF = CHS[c], OFFS[c]
                    T1, T2, T3, part0, col0 = psum_t[c]
                    xt = dat[part0:part0 + 64, col0:col0 + CW]
                    # ---- GRU cell ----
                    wr = slice(part0, part0 + 64)
                    nc.tensor.matmul(T2[:, 0:CW], wihT[wr, 0:128], xt, start=True, stop=False)
                    nc.tensor.matmul(T2[:, 0:CW], whhT[:, 0:128], x[:, :], start=False, stop=True)
                    nc.tensor.matmul(T2[:, CW:2 * CW], wihT[wr, 128:256], xt, start=True, stop=False)
                    nc.tensor.matmul(T2[:, CW:2 * CW], whhT[:, 128:256], x[:, :], start=False, stop=True)
                    if z_bhhn:
                        nc.tensor.matmul(T1[:, CW:2 * CW], whhT[:, 256:384], x[:, :],
                                         start=True, stop=True)
                    else:
                        nc.tensor.matmul(T1[:, CW:2 * CW], whhT[:, 256:384], x[:, :],
                                         start=True, stop=False)
                        nc.tensor.matmul(T1[:, CW:2 * CW], bhhn[0:1, :], ones[0:1, 0:CW],
                                         start=False, stop=True)
                    nc.tensor.matmul(T1[:, 0:CW], wihT[wr, 256:384], xt, start=True, stop=False)
                    if not z_bhhn:
                        nc.tensor.matmul(T1[:, 0:CW], bhhn[0:1, :], ones[0:1, 0:CW],
                                         start=False, stop=False)
                    nc.tensor.matmul(T1[:, 0:CW], whhT[:, 256:384], x[:, :],
                                     start=False, stop=True)
                    emit_filler(FILL_GRU)

                    trz = sbp.tile([128, 2 * CW], BF16, tag="trz")
                    if z_brz:
                        nc.scalar.activation(trz[:, 0:CW], T2[:, 0:CW], AF.Tanh, scale=0.5)
                        nc.scalar.activation(trz[:, CW:2 * CW], T2[:, CW:2 * CW],
                                             AF.Tanh, scale=0.5)
                    else:
                        nc.scalar.activation(trz[:, 0:CW], T2[:, 0:CW], AF.Tanh,
                                             scale=0.5, bias=brzt[:, 0:1])
                        nc.scalar.activation(trz[:, CW:2 * CW], T2[:, CW:2 * CW], AF.Tanh,
                                             scale=0.5, bias=brzt[:, 1:2])
                    tz = trz[:, CW:2 * CW]
                    wge = sbp.tile([128, CW], BF16, tag="wge")
                    nc.vector.tensor_mul(wge[:, :], trz[:, 0:CW], T1[:, CW:2 * CW])
                    npre = sbp.tile([128, CW], BF16, tag="npre")
                    nc.vector.tensor_add(npre[:, :], wge[:, :], T1[:, 0:CW])
                    ng = sbp.tile([128, CW], BF16, tag="ng")
                    nc.scalar.activation(ng[:, :], npre[:, :], AF.Tanh, scale=0.5,
                                         bias=biasn[:, 0:1])
                    # z weights negated on host: tz slot holds tq = tanh(-gz/2),
                    # p = 0.5*m*(1+tq), qneg = (p-1)*x, x_new = p*ng - qneg
                    p = sbp.tile([128, CW], BF16, tag="p")
                    nc.vector.scalar_tensor_tensor(p[:, :], tz, 1.0,
                                                   mrep[:, OFF:OFF + CW],
                                                   ALU.add, ALU.mult)
                    qneg = sbp.tile([128, CW], BF16, tag="qneg")
                    nc.vector.scalar_tensor_tensor(qneg[:, :], p[:, :], 1.0,
                                                   x[:, :], ALU.subtract, ALU.mult)
                    u1 = sbp.tile([128, CW], BF16, tag="u1")
                    nc.vector.tensor_mul(u1[:, :], p[:, :], ng[:, :])
                    nc.vector.tensor_tensor(x[:, :], u1[:, :], qneg[:, :],
                                            ALU.subtract)

            def emit_head(c):
                sbp, psp, x = sbp_[c], psp_[c], xs[c]
                CW, OFF = CHS[c], OFFS[c]
                T3all = ps3.tile([128, BS], F32, tag="T3all")
                hd1 = T3all[0:100, OFF:OFF + CW]
                nc.tensor.matmul(hd1[:, :], z0w0T[:, :], x[:, :], start=True, stop=True)
                hid = sbp.tile([100, CW], BF16, tag="h1")
                nc.scalar.activation(hid[:, :], hd1[:, :], AF.Tanh, bias=z0b0[:, 0:1])
                hd2 = psp.tile([128, CW], F32, tag="T1")
                nc.tensor.matmul(hd2[:, :], z0w1T[:, :], hid[:, :], start=True, stop=True)
                nc.scalar.activation(out_sb[:, OFF:OFF + CW], hd2[:, :],
                                     AF.Identity, bias=z0b1[:, 0:1])

            N = reps * T
            for s_rep in range(N + 1):
                if s_rep < N:
                    emit_flow(0, s_rep)
                if s_rep >= 1:
                    if s_rep == N:
                        emit_head(0)  # overlaps chunk 1's final gru chain
                    emit_gru(1, s_rep - 1)
                if s_rep < N:
                    emit_gru(0, s_rep)
                    emit_flow(1, s_rep)
            emit_head(1)
            nc.sync.dma_start(out=out_dm[:, :], in_=out_sb[:, :])

    nc.compile()
    return nc


def _prep(inputs):
    f = {k: np.asarray(v, dtype=np.float32) for k, v in inputs.items()}
    data, ts = f["data"], f["time_steps"]
    dts = np.concatenate([np.float32([-0.01]), (ts[:-1] - ts[1:])[::-1]]).astype(np.float32)

    shared = {}
    flags = []
    for l in range(2):
        w0, b0 = f[f"f{l}_w0"], f[f"f{l}_b0"]
        w1, b1 = f[f"f{l}_w1"], f[f"f{l}_b1"]
        w2, b2 = f[f"f{l}_w2"], f[f"f{l}_b2"]
        tw = f[f"f{l}_tw"]
        act = slice(0, 64) if l == 0 else slice(64, 128)
        upd = slice(64, 128) if l == 0 else slice(0, 64)

        w0a = w0[:, act]                       # [256, 64]
        w0aT = np.ascontiguousarray(w0a.T)     # [64, 256]
        shared[f"w0aT{l}"] = np.concatenate([w0aT, w0aT], axis=0)  # [128,256] dup
        w1T = np.ascontiguousarray(w1.T)       # [256, 256] (in,out)
        blk = np.empty((128, 512), np.float32)
        for ki in range(2):
            for mj in range(2):
                blk[:, (2 * ki + mj) * 128:(2 * ki + mj) * 128 + 128] = \
                    w1T[ki * 128:(ki + 1) * 128, mj * 128:(mj + 1) * 128]
        shared[f"w1T{l}"] = blk
        ui = np.arange(128)[upd]
        w2s = np.concatenate([w2[ui, :], w2[128 + ui, :]], axis=0)  # [128, 256]
        b2s = np.concatenate([b2[ui], b2[128 + ui]])
        w2sT = np.ascontiguousarray(w2s.T)     # [256, 128]
        shared[f"w2sT{l}"] = np.concatenate([w2sT[0:128, :].reshape(128, 128),
                                             w2sT[128:256, :].reshape(128, 128)], axis=1)
        d0 = np.outer(w0[:, 128], dts) + b0[:, None]       # [256, T]
        shared[f"d0a{l}"] = np.ascontiguousarray(d0[0:128, :])
        shared[f"d0b{l}"] = np.ascontiguousarray(d0[128:256, :])
        tt = np.tanh(np.outer(tw, dts))                    # [256, T]
        tscl = np.ascontiguousarray(tt[0:128][ui, :])    # [64, T]
        tshl = np.ascontiguousarray(tt[128:256][ui, :])  # [64, T]
        ebl = tscl * b2s[0:64, None]
        sb2l = tshl * b2s[64:128, None]
        dup = lambda v: np.ascontiguousarray(np.concatenate([v, v], axis=0))
        shared[f"tsc{l}"] = dup(tscl)
        shared[f"tsh{l}"] = dup(tshl)
        shared[f"eb{l}"] = dup(ebl)
        shared[f"sb2t{l}"] = dup(sb2l)
        b1t = np.stack([b1[0:128], b1[128:256]], axis=1)
        shared[f"b1t{l}"] = np.ascontiguousarray(b1t)
        flags += [bool(np.all(b1 == 0)), bool(np.all(b2 == 0))]

    wih, whh = f["gru_wih"], f["gru_whh"]
    bih, bhh = f["gru_bih"], f["gru_bhh"]
    wihT = np.ascontiguousarray(wih.T)                     # [64, 384]
    shared["wihT"] = np.concatenate([wihT, wihT], axis=0)  # [128, 384]
    shared["whhT"] = np.ascontiguousarray(whh.T)           # [128, 384]
    wihT = shared["wihT"].copy(); whhT = shared["whhT"].copy()
    wihT[:, 128:256] *= -1.0
    whhT[:, 128:256] *= -1.0
    wihT[:, 256:384] *= 2.0
    shared["wihT"] = wihT; shared["whhT"] = whhT
    brz = 0.5 * (bih[0:256] + bhh[0:256])
    brz = np.concatenate([brz[0:128], -brz[128:256]])
    shared["brzt"] = np.ascontiguousarray(np.stack([brz[0:128], brz[128:256]], axis=1))
    shared["biasn"] = np.ascontiguousarray(bih[256:384][:, None])
    shared["bhhn"] = np.ascontiguousarray(bhh[256:384][None, :])
    shared["ones"] = np.ones((128, 512), np.float32)
    shared["z0w0T"] = np.ascontiguousarray(f["z0_w0"].T)   # [128, 100]
    shared["z0b0"] = np.ascontiguousarray(f["z0_b0"][:, None])
    shared["z0w1T"] = np.ascontiguousarray(f["z0_w1"].T)   # [100, 128]
    shared["z0b1"] = np.ascontiguousarray(f["z0_b1"][:, None])
    flags += [bool(np.all(brz == 0)), bool(np.all(bhh[256:384] == 0))]
    # flags order: zb1_0, zb2_0, zb1_1, zb2_1, z_brz, z_bhhn -> reorder
    flags = (flags[0], flags[2], flags[1], flags[3], flags[4], flags[5])

    # data slab [D, T_rev, B] and masks
    arr = np.ascontiguousarray(data.transpose(2, 1, 0)[:, ::-1, :])   # [64, 64, 4096]
    mask = 0.5 * (data[:, ::-1, IN_DIM:].sum(axis=2) > 0).astype(np.float32)  # [B, T] rev
    mask = np.ascontiguousarray(mask.T)                               # [T, B]

    BF = ml_dtypes.bfloat16
    for k in ("wihT", "whhT", "z0w0T", "z0w1T", "bhhn", "ones"):
        shared[k] = shared[k].astype(BF)
    for l in range(2):
        for k in (f"w0aT{l}", f"w1T{l}", f"w2sT{l}"):
            shared[k] = shared[k].astype(BF)

    in_maps = []
    for c in range(NCORES):
        sl = slice(c * BS, (c + 1) * BS)
        ac = arr[:, :, sl]                                            # [64, 64, 512]
        packed = np.empty((128, HT * BS), np.float32)
        packed[0:64, :] = ac[:, 0:HT, :].reshape(64, HT * BS)
        packed[64:128, :] = ac[:, HT:T, :].reshape(64, HT * BS)
        m = {"dat": packed.astype(BF), "msk": np.ascontiguousarray(mask[:, sl]).astype(BF)}
        m.update(shared)
        in_maps.append(m)
    return in_maps, flags


def kernel(**inputs):
    in_maps, flags = _prep(inputs)
    if _CACHE.get("flags") != flags:
        _CACHE["nc"] = _build(flags)
        _CACHE["flags"] = flags
    res = run_bass_kernel_spmd(_CACHE["nc"], in_maps, core_ids=list(range(NCORES)))
    _CACHE["last_res"] = res
    mean = np.empty((B, LAT), np.float32)
    stdp = np.empty((B, LAT), np.float32)
    for c in range(NCORES):
        o = res.results[c]["out"]                 # [128, 512]
        mean[c * BS:(c + 1) * BS] = o[0:LAT, :].T
        stdp[c * BS:(c + 1) * BS] = o[LAT:2 * LAT, :].T
    std = np.logaddexp(0.0, stdp).astype(np.float32)      # softplus
    return mean[None, :, :], std[None, :, :]

